# revision 9
# baseline (speedup 1.0000x reference)
"""Trainium2 Bass kernel for ContextQueryAttn (BiDAF-style trilinear attention).

Computes, per batch b:
    sim = sc[:,None] + sq[None,:] + (ctx*wm) @ query.T          (Lc, Lq)
    sim = where(cmask[:,None] | qmask[None,:], -1e30, sim)
    S   = softmax(sim, axis=-1)   (row softmax over Lq)
    SS  = softmax(sim, axis=0)    (col softmax over Lc)
    A   = S @ query               (Lc, D)
    T   = SS.T @ ctx              (Lq, D)
    B   = S @ T                   (Lc, D)
returns (A, B).

Strategy: data-parallel over batch B=32 across 8 cores (4 batches/core).
All matmuls on PE in float32r (fp22 mantissa, full speed at N>=256).
Softmaxes use no max-subtraction (logits are O(+-10); masked entries hit
exp(-1e30)=0 exactly); normalizers come from ones-columns appended to the
matmul RHS operands; fully-masked rows/cols reproduce the reference's
uniform-softmax semantics via ACT scale/bias folding and a predicated copy.
"""

import os
import numpy as np

import concourse.bass as bass
import concourse.tile as tile
from concourse import bacc, mybir
from concourse.bass_utils import run_bass_kernel_spmd

F32 = mybir.dt.float32
F32R = mybir.dt.float32r
EXP = mybir.ActivationFunctionType.Exp
ALU = mybir.AluOpType

B, LC, LQ, D = 32, 2048, 256, 256
NCORES = 8
BPC = B // NCORES          # batches per core
NCT = LC // 128            # 16 context tiles
NQT = LQ // 128            # 2 query tiles
NKD = D // 128             # 2 contraction chunks over D
NCH = LC // 512            # 4 dotT column chunks
NEG = np.float32(-1e30)

# Transposes in plain f32 (2 cyc/row) by default; f32r (1.5) is an option.
TRANSPOSE_DT = F32R


def _build_kernel(tc, nc, ins, outs):
    import contextlib
    ctx = contextlib.ExitStack()

    sb = lambda name, bufs: ctx.enter_context(
        tc.tile_pool(name=name, bufs=bufs))
    ps_pool = ctx.enter_context(tc.tile_pool(name="ps", bufs=5, space="PSUM"))
    t_pool = ctx.enter_context(tc.tile_pool(name="tps", bufs=1, space="PSUM"))

    p_const = sb("const", 1)
    p_ctx = sb("pctx", 2)
    p_ctxT = sb("pctxT", 2)
    p_PT = sb("pPT", 2)
    p_Pc = sb("pPc", 2)
    p_q = sb("pq", 2)
    p_qwmT = sb("pqwmT", 2)
    p_Tn = sb("pTn", 2)
    p_cm = sb("pcm", 2)
    p_cs = sb("pcs", 2)
    p_vec = sb("pvec", 2)
    p_stage = sb("pstage", 8)

    ident = p_const.tile([128, 128], F32R, name="ident")
    nc.sync.dma_start(out=ident[:], in_=ins["ident"])

    r128 = lambda ap: ap.rearrange("(t p) x -> p t x", p=128)
    v128 = lambda ap: ap.rearrange("(t p) -> p t", p=128)

    for b in range(BPC):
        # ---- loads ----
        ctx_sb = p_ctx.tile([128, NCT, 258], F32R, name="ctx_sb")
        nc.sync.dma_start(out=ctx_sb[:], in_=r128(ins["ctx_ext"][b]))
        q_sb = p_q.tile([128, NQT, 258], F32R, name="q_sb")
        nc.sync.dma_start(out=q_sb[:], in_=r128(ins["query_ext"][b]))
        qwmT_sb = p_qwmT.tile([128, NKD, LQ], F32R, name="qwmT_sb")
        nc.sync.dma_start(out=qwmT_sb[:], in_=r128(ins["qwmT"][b]))

        sqb_sb = p_vec.tile([128, NQT], F32, name="sqb_sb")
        nc.sync.dma_start(out=sqb_sb[:], in_=v128(ins["sq_bias"][b]))
        nbs_sb = p_vec.tile([128, NQT], F32, name="nbs_sb")
        nc.sync.dma_start(out=nbs_sb[:], in_=v128(ins["nbs"][b]))
        qsc_sb = p_vec.tile([128, NQT], F32, name="qsc_sb")
        nc.sync.dma_start(out=qsc_sb[:], in_=v128(ins["q_scale"][b]))
        qf_sb = p_vec.tile([128, NQT], F32, name="qf_sb")
        nc.sync.dma_start(out=qf_sb[:], in_=v128(ins["qf"][b]))
        scb_sb = p_vec.tile([128, NCT], F32, name="scb_sb")
        nc.sync.dma_start(out=scb_sb[:], in_=v128(ins["sc_bias"][b]))
        csc_sb = p_vec.tile([128, NCT], F32, name="csc_sb")
        nc.sync.dma_start(out=csc_sb[:], in_=v128(ins["c_scale"][b]))

        cmaskb_sb = p_cm.tile([128, LC], mybir.dt.uint8, name="cmaskb_sb")
        nc.sync.dma_start(out=cmaskb_sb[:],
                          in_=ins["cmask"][b][None, :].to_broadcast((128, LC)))
        ctxsum_sb = p_cs.tile([128, 258], F32, name="ctxsum_sb")
        nc.sync.dma_start(out=ctxsum_sb[:],
                          in_=ins["ctxsum_ext"][b][None, :].to_broadcast((128, 258)))

        rsrec_sb = p_vec.tile([128, NCT], F32, name="rsrec_sb")
        csrec_sb = p_vec.tile([128, NQT], F32, name="csrec_sb")

        # ---- ctx transposes: ctxT[kd] = ctx[:, kd-block].T  (d on partitions)
        ctxT_sb = p_ctxT.tile([128, NKD, LC], F32R, name="ctxT_sb")
        for kd in range(NKD):
            for g in range(NCH):
                tp = ps_pool.tile([128, 512], F32R, tag="ps", name="tp")
                for j in range(4):
                    ci = 4 * g + j
                    nc.tensor.transpose(
                        out=tp[:, bass.ts(j, 128)],
                        in_=ctx_sb[:, ci, bass.ts(kd, 128)],
                        identity=ident[:])
                nc.scalar.copy(ctxT_sb[:, kd, bass.ts(g, 512)], tp[:])

        # ---- row path: dotT (q, c) -> exp -> P^T, cmask predicated copy
        PT_sb = p_PT.tile([128, NQT, LC], F32R, name="PT_sb")
        for qt in range(NQT):
            for ch in range(NCH):
                dt_ps = ps_pool.tile([128, 512], F32, tag="ps", name="dt_ps")
                for kd in range(NKD):
                    nc.tensor.matmul(
                        dt_ps[:],
                        lhsT=qwmT_sb[:, kd, bass.ts(qt, 128)],
                        rhs=ctxT_sb[:, kd, bass.ts(ch, 512)],
                        start=(kd == 0), stop=(kd == NKD - 1))
                # cmasked columns -> -sq_bias[q], cancelling the exp bias
                # exactly: exp(0)=1 (uniform row), incl. qmasked rows where
                # +1e30 + (-1e30) = 0.
                nc.vector.copy_predicated(
                    out=dt_ps[:], mask=cmaskb_sb[:, bass.ts(ch, 512)],
                    data=nbs_sb[:, qt:qt + 1].to_broadcast((128, 512)))
                nc.scalar.activation(
                    PT_sb[:, qt, bass.ts(ch, 512)], dt_ps[:], EXP,
                    bias=sqb_sb[:, qt:qt + 1])

        # ---- col path: dot (c, q) -> exp -> Pc; T accumulation
        T_ps = [t_pool.tile([128, 258], F32, name=f"T_ps{qt}") for qt in range(NQT)]
        Pc_sb = p_Pc.tile([128, NCT, LQ], F32R, name="Pc_sb")
        for ci in range(NCT):
            dps = ps_pool.tile([128, LQ], F32, tag="ps", name="dps")
            for kd in range(NKD):
                nc.tensor.matmul(
                    dps[:],
                    lhsT=ctxT_sb[:, kd, bass.ts(ci, 128)],
                    rhs=qwmT_sb[:, kd, :],
                    start=(kd == 0), stop=(kd == NKD - 1))
            nc.scalar.activation(
                Pc_sb[:, ci, :], dps[:], EXP,
                bias=scb_sb[:, ci:ci + 1], scale=csc_sb[:, ci:ci + 1])
            for qt in range(NQT):
                nc.tensor.matmul(
                    T_ps[qt][:],
                    lhsT=Pc_sb[:, ci, bass.ts(qt, 128)],
                    rhs=ctx_sb[:, ci, :],
                    start=(ci == 0), stop=(ci == NCT - 1))

        # ---- A path (needs P^T finalized)
        for ci in range(NCT):
            a_ps = ps_pool.tile([128, 258], F32, tag="ps", name="a_ps")
            for qt in range(NQT):
                nc.tensor.matmul(
                    a_ps[:],
                    lhsT=PT_sb[:, qt, bass.ts(ci, 128)],
                    rhs=q_sb[:, qt, :],
                    start=(qt == 0), stop=(qt == NQT - 1))
            nc.vector.reciprocal(rsrec_sb[:, ci:ci + 1], a_ps[:, 256:257])
            a_st = p_stage.tile([128, 256], F32, tag="ast", name="a_st")
            nc.scalar.mul(a_st[:], a_ps[:, 0:256], rsrec_sb[:, ci:ci + 1])
            nc.sync.dma_start(out=outs["A"][b, bass.ts(ci, 128), :], in_=a_st[:])

        # ---- T finalize: blend qmask + normalize
        Tn_sb = p_Tn.tile([128, NQT, 256], F32R, name="Tn_sb")
        for qt in range(NQT):
            nc.vector.tensor_scalar_mul(
                T_ps[qt][:], T_ps[qt][:], qsc_sb[:, qt:qt + 1])
            nc.vector.scalar_tensor_tensor(
                out=T_ps[qt][:], in0=ctxsum_sb[:], scalar=qf_sb[:, qt:qt + 1],
                in1=T_ps[qt][:], op0=ALU.mult, op1=ALU.add)
            nc.vector.reciprocal(csrec_sb[:, qt:qt + 1], T_ps[qt][:, 256:257])
            nc.scalar.mul(Tn_sb[:, qt, :], T_ps[qt][:, 0:256],
                          csrec_sb[:, qt:qt + 1])

        # ---- Bmat = S @ T
        for ci in range(NCT):
            b_ps = ps_pool.tile([128, 256], F32, tag="ps", name="b_ps")
            for qt in range(NQT):
                nc.tensor.matmul(
                    b_ps[:],
                    lhsT=PT_sb[:, qt, bass.ts(ci, 128)],
                    rhs=Tn_sb[:, qt, :],
                    start=(qt == 0), stop=(qt == NQT - 1))
            b_st = p_stage.tile([128, 256], F32, tag="bst", name="b_st")
            nc.vector.tensor_scalar_mul(b_st[:], b_ps[:], rsrec_sb[:, ci:ci + 1])
            nc.sync.dma_start(out=outs["Bm"][b, bass.ts(ci, 128), :], in_=b_st[:])

    ctx.close()


def build_program():
    nc = bacc.Bacc("TRN2", target_bir_lowering=False, debug=False,
                   num_devices=NCORES)
    ins = {
        "ctx_ext": nc.dram_tensor("ctx_ext", [BPC, LC, 258], F32R,
                                  kind="ExternalInput").ap(),
        "query_ext": nc.dram_tensor("query_ext", [BPC, LQ, 258], F32R,
                                    kind="ExternalInput").ap(),
        "qwmT": nc.dram_tensor("qwmT", [BPC, D, LQ], F32R,
                               kind="ExternalInput").ap(),
        "sq_bias": nc.dram_tensor("sq_bias", [BPC, LQ], F32,
                                  kind="ExternalInput").ap(),
        "q_scale": nc.dram_tensor("q_scale", [BPC, LQ], F32,
                                  kind="ExternalInput").ap(),
        "qf": nc.dram_tensor("qf", [BPC, LQ], F32, kind="ExternalInput").ap(),
        "sc_bias": nc.dram_tensor("sc_bias", [BPC, LC], F32,
                                  kind="ExternalInput").ap(),
        "c_scale": nc.dram_tensor("c_scale", [BPC, LC], F32,
                                  kind="ExternalInput").ap(),
        "cmask": nc.dram_tensor("cmask", [BPC, LC], mybir.dt.uint8,
                                kind="ExternalInput").ap(),
        "ctxsum_ext": nc.dram_tensor("ctxsum_ext", [BPC, 258], F32,
                                     kind="ExternalInput").ap(),
        "nbs": nc.dram_tensor("nbs", [BPC, LQ], F32,
                              kind="ExternalInput").ap(),
        "ident": nc.dram_tensor("ident", [128, 128], F32R,
                                kind="ExternalInput").ap(),
    }
    outs = {
        "A": nc.dram_tensor("A", [BPC, LC, D], F32, kind="ExternalOutput").ap(),
        "Bm": nc.dram_tensor("Bm", [BPC, LC, D], F32, kind="ExternalOutput").ap(),
    }
    with tile.TileContext(nc) as tc:
        _build_kernel(tc, nc, ins, outs)
    nc.compile()
    return nc


def host_prep(context, query, context_mask, query_mask, w0):
    """Host-side preprocessing: shard + build auxiliary tensors (all O(B*L*D))."""
    f = np.float32
    context = np.ascontiguousarray(context, dtype=f)
    query = np.ascontiguousarray(query, dtype=f)
    w0 = np.asarray(w0, dtype=f)
    wc, wq, wm = w0[:D], w0[D:2 * D], w0[2 * D:]
    cf = context_mask.astype(f)
    qf = query_mask.astype(f)
    sc = context @ wc                      # (B, LC)
    sq = query @ wq                        # (B, LQ)
    qwmT = np.ascontiguousarray((query * wm).transpose(0, 2, 1))
    ones_c = np.ones((B, LC, 1), f)
    ones_q = np.ones((B, LQ, 1), f)
    zc = np.zeros((B, LC, 1), f)
    zq = np.zeros((B, LQ, 1), f)
    ctx_ext = np.ascontiguousarray(np.concatenate([context, ones_c, zc], -1))
    query_ext = np.ascontiguousarray(np.concatenate([query, ones_q, zq], -1))
    ctxsum_ext = np.concatenate(
        [context.sum(1, dtype=f), np.full((B, 1), LC, f),
         np.zeros((B, 1), f)], -1)
    q_scale = (1.0 - qf).astype(f)
    sq_bias = (q_scale * sq + qf * NEG).astype(f)
    c_scale = (1.0 - cf).astype(f)
    sc_bias = (c_scale * sc + cf * NEG).astype(f)

    full = {
        "ctx_ext": ctx_ext, "query_ext": query_ext, "qwmT": qwmT,
        "sq_bias": sq_bias, "nbs": -sq_bias, "q_scale": q_scale, "qf": qf,
        "sc_bias": sc_bias, "c_scale": c_scale,
        "cmask": cf.astype(np.uint8),
        "ctxsum_ext": ctxsum_ext,
    }
    const = {"ident": np.eye(128, dtype=f)}
    in_maps = []
    for c in range(NCORES):
        sl = slice(c * BPC, (c + 1) * BPC)
        m = {k: np.ascontiguousarray(v[sl]) for k, v in full.items()}
        m.update(const)
        in_maps.append(m)
    return in_maps


_cached_nc = None


def get_program():
    global _cached_nc
    if _cached_nc is None:
        _cached_nc = build_program()
    return _cached_nc


def run_on_hw(in_maps, **kwargs):
    nc = get_program()
    return run_bass_kernel_spmd(nc, in_maps, core_ids=list(range(NCORES)),
                                **kwargs)


def kernel(context, query, context_mask, query_mask, w0):
    in_maps = host_prep(context, query, context_mask, query_mask, w0)
    res = run_on_hw(in_maps)
    A = np.concatenate([res.results[c]["A"] for c in range(NCORES)], 0)
    Bm = np.concatenate([res.results[c]["Bm"] for c in range(NCORES)], 0)
    return A, Bm
